# revision 19
# baseline (speedup 1.0000x reference)
"""Trainium2 kernel for per-subject linear heads (moe_routing).

Computes out[i] = x[i] @ W[subject_ids[i]] + b[subject_ids[i]] for
B=256, D=2048, S=8 subjects, OUT=1000.

Sharding: expert-parallel — core s owns subject s. Each core reads only
its own (2048, 1000) weight slice from HBM, so the total weight traffic
across the chip is W read exactly once (vs 8x for batch-data-parallel
with a replicated table). Samples are grouped by subject on the host,
padded to a fixed capacity C, and fed to an SPMD Bass/Tile kernel;
outputs are scattered back to the original order.

The harness gate is rel_err < 2e-2; x/W/bias are converted to bf16 on
the host, which halves the dominant HBM traffic (W: 8.2 MB -> 4.1 MB
per core) and runs the PE in single-pass bf16 instead of the fp32
LOW/HIGH double-pass. Resulting rel err ~2e-3.

Schedule (per core), tuned against perfetto traces:
- Both HWDGE rings stream 10 chunk blocks (5 per ring, alternating in
  k order).  The two x halves are EMBEDDED in the first chunk of each
  ring: x alone needs 128 scatter descriptors of <1 KB which measurably
  stall the stream ramp; fused into the weight blocks the same bytes
  ride 4.8 KB descriptors.  Every block is contiguous in DRAM (strided
  chunk reads measured ~2.5x slower).  Chunks shrink toward the end of
  the stream ([x+2,2,2,1,1] k-tiles per ring) so the last dependency is
  small and the final matmul burst short.
- Chunks 9 and 10 reuse HWDGE completion lanes 0/1 — legal because a W
  load has no data wait, so the lane-reuse wait is its single wait.
- The y writes are FIRE-AND-FORGET: they go out on dedicated SWDGE
  lanes whose completion semaphores are deliberately neither drained
  nor cleared by the exit sequence.  The ~0.5 us transfer and ~2 us
  HBM-write receipt then overlap the fixed ~6.5 us compiler epilogue
  (per-engine semaphore-clear chains) instead of preceding it.  Their
  sems are left dirty for a NEFF re-execution, which is safe: nothing
  waits on those lanes (each has a single user).
- The bias rank-1 update (ones row carried as an extra k-slot of x,
  times the [1, OUT] bias) is emitted mid-stream — PSUM accumulation
  order is free and the SWDGE bias transfer has long landed.
- The two n-tiles accumulate in separate PSUM banks but stream
  concurrently on disjoint PE column halves (tile_position). Separate
  banks keep the tail copies independent: n=0 copies on ACT while n=1
  copies on DVE.
- Spin matmuls on a scratch tile keep the PE busy from kernel start so
  the HAM clock-gate reaches 2.4 GHz before, and stays there between,
  the real matmul bursts.

This walrus build rejects any instruction with more than one sync
wait; every instruction here needs at most one: per-half warm-up
matmuls absorb the two x waits, a throwaway DVE op absorbs the
GpSimd-port wait DVE would otherwise add to its PSUM copy, each
chunk's first matmul waits only on its own chunk, each y DMA waits
only on its own copy.
"""

import numpy as np
import ml_dtypes

import concourse.bass as bass
import concourse.mybir as mybir
import concourse.tile as tile
from concourse.bass_utils import run_bass_kernel_spmd
from concourse.vector_clock import ScopedClock, VectorClock

B = 256
D = 2048
S = 8
OUT = 1000
P = 128
KO = D // P          # 16 k-tiles of 128
NT = 500             # psum n-tile (<= 512 fp32 / bank), 2 tiles cover OUT
XA = 9               # x k-slots embedded in chunk 0 (ring 0)
XB = KO + 1 - XA     # x k-slots embedded in chunk 1 (ring 1), incl. ones row

# W chunks in consumption (k) order; rings alternate.  k-tiles per
# chunk: chunks 0/1 also carry the x halves.  The sync ring (even
# chunks) carries 9 k-tiles vs the scalar ring's 7: the qACT HWDGE ring
# measurably starts its stream ~1.5 us later than qSP, so equal byte
# loads would leave qACT finishing late; this split makes both rings
# finish together.  The 9th W DMA reuses completion lane 0 — legal, its
# lane wait is its only wait.
CHUNK_K = [2, 2, 2, 2, 2, 2, 2, 1, 1]
N_CHUNKS = len(CHUNK_K)
assert sum(CHUNK_K) == KO

SPINS_PRE = 6        # PE warm-up matmuls before the real stream
SPIN_N = 512         # spin matmul free dim
# Spins after each chunk's burst (clock keep-alive between bursts).
GAP_SPINS = [8, 3, 3, 3, 3, 3, 2, 0, 0]
BF16 = mybir.dt.bfloat16

TRACE = False        # set by test harness to collect an NTFF profile
LAST_RESULTS = None  # BassKernelResults of the most recent run

_nc_cache = {}

# Logical-processor indices in the Tile vector clock (probed): 0=Pool,
# 1=ACT, 2=PE, 3=DVE, 11..18=DMASW0-7, 19..26=DMAHW0-7.
_DMASW0_PROC = 11


class _FastExitTileContext(tile.TileContext):
    """TileContext with a single-wait, barrier-free, fire-and-forget exit.

    This walrus build rejects instructions with >1 sync wait, and the
    stock exit (one Drain waiting on every semaphore + two all-engine
    EVSEM-butterfly barriers) both violates that and costs ~8 us. Here
    SP emits one drain per logical processor (each <=1 wait), then
    hands off to GpSimd via a fresh semaphore; GpSimd resets the DMA
    queues and clears the Tile semaphores (required so a re-execution
    of the NEFF starts from zeroed sems). By the time SP's drains have
    observed every semaphore at its final value, every engine has
    retired its last instruction, so the butterfly barriers are
    unnecessary.

    Processors listed in ``skip_procs`` (the fire-and-forget y writes)
    are neither drained nor dma-reset/cleared: their DMAs are still in
    flight at exit on purpose — the data lands under the compiler's
    fixed epilogue, and nothing ever waits on those lanes, so the dirty
    semaphore value is harmless even across NEFF re-executions.
    """

    skip_procs: frozenset = frozenset()

    def _drain_and_barrier(self, tick_clock, wait_clock):
        nc = self.nc
        gc = tick_clock.global_clock
        n = len(gc)
        last = None
        for i in range(n):
            if gc[i] <= 0 or i in self.skip_procs:
                continue
            vec = [0] * n
            vec[i] = gc[i]
            d = nc.sync.drain()
            wait_clock.add_sem_waits(d.ins, ScopedClock({None: VectorClock(vec)}))
            last = d

        assert self.sems is not None
        popped = nc._tile_sem_poison_stack.pop()
        assert popped is self._sem_poison
        sems = [
            s for k, s in self.sems.allocated().items() if k not in self.skip_procs
        ]
        if last is not None:
            handoff = nc.alloc_semaphore(name="exit_handoff")
            last.then_inc(handoff, 1)
            nc.gpsimd.wait_ge(handoff, 1)
            nc.clear_and_free_semaphores(sems)
            nc.gpsimd.sem_clear(handoff)
            nc.release_semaphore(handoff)
        else:
            nc.clear_and_free_semaphores(sems)


def _build(C):
    """Per-core program: y[C, OUT] = x @ W + bias (k-tiled, bf16).

    Inputs (all bf16, per-partition-contiguous DRAM blocks):
      wc0  : [P, XA*C + k0*OUT]   x slots 0..8 then W k-tiles 0..k0-1
      wc1  : [P, XB*C + k1*OUT]   x slots 9..16 (incl ones row), W k-tiles
      wc2+ : [P, k_i*OUT]         remaining W k-tile blocks
      bias : [1, OUT]
    where x slot s holds x_subject[c, s*P + p] (slot KO is all-ones).
    """
    nc = bass.Bass(enable_partition_id=False)
    sizes = []
    for c, k in enumerate(CHUNK_K):
        extra = XA * C if c == 0 else (XB * C if c == 1 else 0)
        sizes.append(extra + k * OUT)
    wcs = [
        nc.dram_tensor(f"wc{c}", [P, sz], BF16, kind="ExternalInput")
        for c, sz in enumerate(sizes)
    ]
    bias = nc.dram_tensor("bias", [1, OUT], BF16, kind="ExternalInput")
    y = nc.dram_tensor("y", [C, OUT], mybir.dt.float32, kind="ExternalOutput")

    m_tiles = [(m0, min(P, C - m0)) for m0 in range(0, C, P)]
    ko_lo = [sum(CHUNK_K[:c]) for c in range(N_CHUNKS)]
    rings = [nc.sync, nc.scalar]

    # y writes ride SWDGE lanes 1..N (bias is SWDGE lane 0); mark them
    # fire-and-forget for the exit sequence.
    n_y = 2 * len(m_tiles)
    ctx_cls = type(
        "_Ctx", (_FastExitTileContext,), {
            "skip_procs": frozenset(
                range(_DMASW0_PROC + 1, _DMASW0_PROC + 1 + n_y)
            )
        },
    )

    with ctx_cls(nc) as tc:
        with (
            tc.tile_pool(name="wpool", bufs=N_CHUNKS) as wpool,
            tc.tile_pool(name="bpool", bufs=1) as bpool,
            tc.tile_pool(name="spool", bufs=1) as spool,
            tc.tile_pool(name="opool", bufs=4) as opool,
            tc.tile_pool(name="psum", bufs=1, space="PSUM") as psum_pool,
        ):
            # PE warm-up scratch: memset by GpSimd so the first spin
            # matmul's only wait is the GpSimd semaphore.
            scratch = spool.tile([P, SPIN_N], BF16)
            nc.gpsimd.memset(scratch[:], 0.0)

            # The tiny bias goes over SWDGE; the HWDGE rings carry only
            # the fused x/W chunk stream.
            b_tile = bpool.tile([1, OUT], BF16)
            nc.gpsimd.dma_start(b_tile[:], bias[:])

            w_tiles = []
            for c, sz in enumerate(sizes):
                wt = wpool.tile([P, sz], BF16)
                rings[c % 2].dma_start(wt[:], wcs[c][:])
                w_tiles.append(wt)

            def x_slot(s, m0, mc):
                """lhsT AP for x k-slot s, batch rows m0..m0+mc."""
                if s < XA:
                    return w_tiles[0][:, s * C + m0 : s * C + m0 + mc]
                s -= XA
                return w_tiles[1][:, s * C + m0 : s * C + m0 + mc]

            def w_slice(ko, lo, hi):
                c = max(i for i in range(N_CHUNKS) if ko_lo[i] <= ko)
                base = (XA * C if c == 0 else (XB * C if c == 1 else 0)) + (
                    ko - ko_lo[c]
                ) * OUT
                return w_tiles[c][:, base + lo : base + hi]

            # Two PSUM banks per m-tile (one per n-tile): the PE streams
            # both concurrently on disjoint column halves via
            # tile_position, and the tail copies stay independent.
            col_tiled = all(mc <= 64 for _, mc in m_tiles)
            psums = {}
            tilepos = {}
            for mi, (m0, mc) in enumerate(m_tiles):
                if col_tiled:
                    bank0 = psum_pool.tile(
                        [P, NT], mybir.dt.float32, name=f"psum_{mi}_0"
                    )
                    bank1 = psum_pool.tile(
                        [P, NT], mybir.dt.float32, name=f"psum_{mi}_1"
                    )
                    psums[(mi, 0)] = bank0[0:mc]
                    psums[(mi, 1)] = bank1[64 : 64 + mc]
                    tilepos[(mi, 0)] = (0, 0)
                    tilepos[(mi, 1)] = (0, 64)
                else:
                    for n in range(2):
                        psums[(mi, n)] = psum_pool.tile(
                            [mc, NT], mybir.dt.float32, name=f"psum_{mi}_{n}"
                        )
                        tilepos[(mi, n)] = None
            spin_ps = psum_pool.tile([1, SPIN_N], mybir.dt.float32, name="spin_ps")

            def spin(k):
                for _ in range(k):
                    nc.tensor.matmul(
                        spin_ps[:, :],
                        scratch[:, 0:1],
                        scratch[:, :SPIN_N],
                        start=True,
                        stop=True,
                    )

            spin(SPINS_PRE // 2)
            # Per-half warm-up matmuls absorb the two x waits (= chunk
            # 0/1 sems; scratch has no DMA dependency), so later matmuls
            # each need only their own chunk/bias wait.
            warm = psum_pool.tile([1, C], mybir.dt.float32, name="warm")
            nc.tensor.matmul(
                warm[:, :], scratch[:, 0:1], w_tiles[0][:, 0:C], start=True, stop=True
            )
            nc.tensor.matmul(
                warm[:, :], scratch[:, 0:1], w_tiles[1][:, 0:C], start=True, stop=True
            )
            spin(SPINS_PRE - SPINS_PRE // 2)
            # DVE absorber: DVE in 2-port mode locks GpSimd out of the
            # SWDGE descriptor rings, so the scheduler orders DVE ops
            # after outstanding GpSimd work; soak that up now so the
            # tail n=1 PSUM copy needs only its PE wait.
            dve_warm = spool.tile([1, 1], BF16)
            nc.vector.tensor_copy(dve_warm[:], b_tile[0:1, 0:1])

            # Chunk bursts in arrival order. start=True opens each
            # accumulation group on k-tile 0, stop=True closes it on
            # k-tile KO-1. The bias rank-1 update is emitted after
            # chunk 3's burst.
            for c, k in enumerate(CHUNK_K):
                for ko in range(ko_lo[c], ko_lo[c] + k):
                    for mi, (m0, mc) in enumerate(m_tiles):
                        lhsT = x_slot(ko, m0, mc)
                        for n in range(2):
                            nc.tensor.matmul(
                                psums[(mi, n)][:, :],
                                lhsT,
                                w_slice(ko, n * NT, (n + 1) * NT),
                                start=(ko == 0),
                                stop=(ko == KO - 1),
                                tile_position=tilepos[(mi, n)],
                            )
                if c == 3:
                    for mi, (m0, mc) in enumerate(m_tiles):
                        ones = x_slot(KO, m0, mc)[0:1]
                        for n in range(2):
                            nc.tensor.matmul(
                                psums[(mi, n)][:, :],
                                ones,
                                b_tile[0:1, n * NT : (n + 1) * NT],
                                start=False,
                                stop=False,
                                tile_position=tilepos[(mi, n)],
                            )
                spin(GAP_SPINS[c])

            # Tail: n=0 copy on ACT, n=1 copy on DVE (concurrent), then
            # fire-and-forget SWDGE y writes (one per copy; each waits
            # only on its own copy; sems never drained — see class doc).
            for mi, (m0, mc) in enumerate(m_tiles):
                ot0 = opool.tile([mc, NT], mybir.dt.float32)
                nc.scalar.copy(ot0[:], psums[(mi, 0)][:])
                ot1 = opool.tile([mc, NT], mybir.dt.float32)
                nc.vector.tensor_copy(ot1[:], psums[(mi, 1)][:])
                nc.gpsimd.dma_start(y[m0 : m0 + mc, 0:NT], ot0[:])
                nc.gpsimd.dma_start(y[m0 : m0 + mc, NT : 2 * NT], ot1[:])
    return nc


def _capacity(max_count):
    c = 48
    while c < max_count:
        c *= 2
    return c


def kernel(x, subject_ids, W, b):
    global LAST_RESULTS
    x = np.ascontiguousarray(np.asarray(x, dtype=np.float32))
    sid = np.asarray(subject_ids).astype(np.int64)
    W = np.ascontiguousarray(np.asarray(W, dtype=np.float32))
    b = np.asarray(b, dtype=np.float32)

    groups = [np.nonzero(sid == s)[0] for s in range(S)]
    C = _capacity(max((len(g) for g in groups), default=1))

    key = (C, SPINS_PRE, tuple(GAP_SPINS), tuple(CHUNK_K))
    if key not in _nc_cache:
        _nc_cache[key] = _build(C)
    nc = _nc_cache[key]

    # W_perm[s, p, ko*OUT + n] = W[s, ko*P + p, n]; chunks are sliced
    # out as contiguous blocks (strided DRAM reads are ~2.5x slower).
    W_perm = np.ascontiguousarray(
        W.reshape(S, KO, P, OUT).transpose(0, 2, 1, 3).astype(ml_dtypes.bfloat16)
    ).reshape(S, P, KO * OUT)
    b_bf = b.astype(ml_dtypes.bfloat16)
    ko_lo = [sum(CHUNK_K[:c]) for c in range(N_CHUNKS)]

    in_maps = []
    for s in range(S):
        idx = groups[s]
        xs = np.zeros((C, D), dtype=np.float32)
        xs[: len(idx)] = x[idx]
        # xT[p, slot, c] = xs[c, slot*P + p]; extra all-ones slot (bias)
        xT = np.empty((P, KO + 1, C), dtype=ml_dtypes.bfloat16)
        xT[:, :KO, :] = xs.T.reshape(KO, P, C).transpose(1, 0, 2).astype(
            ml_dtypes.bfloat16
        )
        xT[:, KO, :] = 1.0
        im = {"bias": b_bf[s : s + 1]}
        for c, k in enumerate(CHUNK_K):
            blk = W_perm[s, :, ko_lo[c] * OUT : (ko_lo[c] + k) * OUT]
            if c == 0:
                blk = np.concatenate([xT[:, :XA, :].reshape(P, XA * C), blk], axis=1)
            elif c == 1:
                blk = np.concatenate([xT[:, XA:, :].reshape(P, XB * C), blk], axis=1)
            im[f"wc{c}"] = np.ascontiguousarray(blk)
        in_maps.append(im)

    LAST_RESULTS = run_bass_kernel_spmd(
        nc, in_maps, core_ids=list(range(S)), trace=TRACE
    )

    out = np.zeros((B, OUT), dtype=np.float32)
    for s in range(S):
        idx = groups[s]
        out[idx] = LAST_RESULTS.results[s]["y"][: len(idx)]
    return out
